# revision 8
# baseline (speedup 1.0000x reference)
"""Trainium2 Bass kernel for nn_NodeLevelContrastiveLoss.

Per-graph contrastive loss:
    s_hat = l2norm(student.reshape(G, Na, D))         # [G, Na, D]
    t_hat = l2norm(teacher.reshape(G, No, D))         # [G, No, D]
    t_al  = t_hat[g, kept[g, i]]                      # [G, Na, D]
    scores = (s_hat @ t_al^T) / T                     # [G, Na, Na]
    loss = mean_g mean_i ( logsumexp_j scores[g,i,j] - scores[g,i,i] )

Sharding: data-parallel over graphs, 32 graphs per core on 8 cores.
Each core returns sum_i (lse_i - diag_i) over its 32*256 rows; the host
sums the 8 partials and divides by G*Na.

Device-side strategy per core (G=32 graphs, 64 "bands" of 128 student rows):
  - only the kept teacher rows leave HBM: dma_gather with 4 SWDGE queues,
    16 gathers of 512 rows each (multi-queue parallelizes Q7 descgen).
  - s is DMA-loaded with an fp32->bf16 cast on the SWDGE path (host
    pre-relayouts s to partition-major so each partition is contiguous).
  - row norms from the bf16 data (consistent rounding with the matmul
    operands): ACT Square -> DVE segmented reduce -> ACT Sqrt(scale=TEMP)
    -> DVE reciprocal.
  - t normalized+cast per band on DVE (bf16 tensor_scalar, 2x mode); s stays
    raw, its inverse norm rides the Exp activation scale and the diag fixup.
  - per graph: 4 PE transposes into ONE psum bank, ONE batched DVE copy to
    SBUF, 2 bf16 matmuls, 2 ACT Exp ops with accum_out as the softmax
    denominator.
  - diag: batched bf16 s*t_hat elementwise + segmented reduce.
"""

import functools

import ml_dtypes
import numpy as np

import concourse.bacc as bacc
import concourse.tile as tile
from concourse import mybir
from concourse.bass_utils import run_bass_kernel_spmd

F32 = mybir.dt.float32
BF16 = mybir.dt.bfloat16
I16 = mybir.dt.int16
AF = mybir.ActivationFunctionType
AX = mybir.AxisListType

N_CORES = 8
NA = 256            # student rows (augmented nodes) per graph
NO = 512            # teacher rows (original nodes) per graph
D = 128
TEMP = 0.1          # temperature; scores get a 1/TEMP factor
NQ = 4              # SWDGE queues
GIDX = 512          # rows per dma_gather


def _emit(nc, G):
    """Emit the per-core kernel for G graphs into Bass object nc."""
    B = G * NA // 128                 # bands of 128 student rows
    CHB = 16 if B % 16 == 0 else B    # bands per chunk
    NCHUNK = B // CHB
    NGPC = CHB * 128 // GIDX          # gathers per chunk

    # s is host-relayouted to [128, B, 128] (partition-major, contiguous per
    # partition) so the SWDGE cast-load emits few descriptors.
    s = nc.dram_tensor("s", [128, B, D], F32, kind="ExternalInput")
    t = nc.dram_tensor("t", [G * NO, D], F32, kind="ExternalInput")
    idx = nc.dram_tensor("idx", [128, B * 8], I16, kind="ExternalInput")
    ident = nc.dram_tensor("ident", [128, 128], BF16, kind="ExternalInput")
    out = nc.dram_tensor("out", [1, 1], F32, kind="ExternalOutput")

    with tile.TileContext(nc) as tc:
        with (
            tc.tile_pool(name="consts", bufs=1) as consts,
            tc.tile_pool(name="big", bufs=3) as big,
            tc.tile_pool(name="stats", bufs=1) as stats,
            tc.tile_pool(name="chk", bufs=2) as chk,
            tc.tile_pool(name="mm", bufs=3) as mm,
            tc.tile_pool(name="ptp", bufs=3, space="PSUM") as ptp,
            tc.tile_pool(name="psc", bufs=4, space="PSUM") as psc,
            tc.tile_pool(name="pfi", bufs=1, space="PSUM") as pfi,
        ):
            idx_sb = consts.tile([128, B * 8], I16)
            nc.sync.dma_start(out=idx_sb, in_=idx[:])
            ident_sb = consts.tile([128, 128], BF16)
            nc.sync.dma_start(out=ident_sb, in_=ident[:])
            ones_sb = consts.tile([128, 1], F32)
            nc.vector.memset(ones_sb, 1.0)

            SINV = stats.tile([128, B], F32)
            DIAG = stats.tile([128, B], F32)
            SSUM = stats.tile([128, B], F32)
            TINV = stats.tile([128, B], F32)

            for c in range(NCHUNK):
                bs = slice(c * CHB, (c + 1) * CHB)

                # s chunk: SWDGE load with fp32 -> bf16 cast
                s_bf = big.tile([128, CHB, 128], BF16, tag="sbf")
                nc.gpsimd.dma_start(out=s_bf, in_=s[:, bs, :])

                # t chunk: multi-queue gathers of GIDX rows each
                t_raw = big.tile([128, CHB, 128], F32, tag="traw")
                gb = GIDX // 128  # bands per gather
                for k in range(NGPC):
                    col0 = c * CHB * 8 + k * (GIDX // 16)
                    nc.gpsimd.dma_gather(
                        out_ap=t_raw[:, k * gb:(k + 1) * gb, :],
                        in_ap=t[:],
                        idxs_ap=idx_sb[:, col0:col0 + GIDX // 16],
                        num_idxs=GIDX,
                        num_idxs_reg=GIDX,
                        elem_size=D,
                        single_packet=False,
                        queue_num=k % NQ,
                    )
                t_bf = big.tile([128, CHB, 128], BF16, tag="tbf")
                nc.vector.tensor_copy(t_bf, t_raw)

                # row sum-of-squares from the bf16 data
                sq = big.tile([128, CHB, 128], BF16, tag="sq")
                nc.scalar.square(sq, s_bf)
                ss = chk.tile([128, 2, CHB], F32, tag="ss")
                nc.vector.reduce_sum(ss[:, 0, :], sq, axis=AX.X)
                sq2 = big.tile([128, CHB, 128], BF16, tag="sq")
                nc.scalar.square(sq2, t_bf)
                nc.vector.reduce_sum(ss[:, 1, :], sq2, axis=AX.X)

                # inv = sqrt(1/T)/|row|: Sqrt(TEMP * ss) then reciprocal
                rs = chk.tile([128, 2, CHB], F32, tag="rs")
                nc.scalar.activation(rs, ss, AF.Sqrt, scale=float(TEMP))
                nc.vector.reciprocal(SINV[:, bs], rs[:, 0, :])
                nc.vector.reciprocal(TINV[:, bs], rs[:, 1, :])

                # normalize t rows (bf16 in/out, 2x DVE mode)
                t_nrm = big.tile([128, CHB, 128], BF16, tag="tnrm")
                for j in range(CHB):
                    nc.vector.tensor_scalar_mul(
                        t_nrm[:, j, :], in0=t_bf[:, j, :],
                        scalar1=TINV[:, c * CHB + j:c * CHB + j + 1],
                    )

                # diagonal dots: sum_d s*t_hat (bf16), then * sinv
                dm = big.tile([128, CHB, 128], BF16, tag="dm")
                nc.vector.tensor_mul(dm, s_bf, t_nrm)
                dsum = chk.tile([128, CHB], F32, tag="dsum")
                nc.vector.reduce_sum(dsum, dm, axis=AX.X)
                nc.vector.tensor_mul(DIAG[:, bs], dsum, SINV[:, bs])

                # per graph: 4 transposes -> one psum bank -> one batched
                # copy -> 2 matmuls -> 2 exp+accum
                for gg in range(CHB // 2):
                    pt = ptp.tile([128, 512], BF16, tag="pt")
                    nc.tensor.transpose(pt[:, 0:128], s_bf[:, 2 * gg, :], ident_sb)
                    nc.tensor.transpose(pt[:, 128:256], s_bf[:, 2 * gg + 1, :], ident_sb)
                    nc.tensor.transpose(pt[:, 256:384], t_nrm[:, 2 * gg, :], ident_sb)
                    nc.tensor.transpose(pt[:, 384:512], t_nrm[:, 2 * gg + 1, :], ident_sb)
                    st = mm.tile([128, 512], BF16, tag="st")
                    nc.vector.tensor_copy(st, pt)
                    for m in range(2):
                        b = c * CHB + 2 * gg + m
                        ps = psc.tile([128, 256], F32, tag="ps")
                        nc.tensor.matmul(
                            ps, lhsT=st[:, m * 128:(m + 1) * 128],
                            rhs=st[:, 256:512], start=True, stop=True,
                        )
                        eo = mm.tile([128, 256], BF16, tag="eo")
                        nc.scalar.activation(
                            eo, ps, AF.Exp,
                            scale=SINV[:, b:b + 1],
                            accum_out=SSUM[:, b:b + 1],
                        )

            # loss partial = sum over all rows of (ln(SSUM) - DIAG)
            LNS = stats.tile([128, B], F32)
            nc.scalar.activation(LNS, SSUM, AF.Ln)
            LL = stats.tile([128, B], F32)
            nc.vector.tensor_sub(LL, LNS, DIAG)
            rt = stats.tile([128, 1], F32)
            nc.vector.reduce_sum(rt, LL, axis=AX.X)
            pfin = pfi.tile([1, 1], F32, tag="fin")
            nc.tensor.matmul(pfin, lhsT=rt, rhs=ones_sb, start=True, stop=True)
            osb = stats.tile([1, 1], F32)
            nc.vector.tensor_copy(osb, pfin)
            nc.sync.dma_start(out=out[:], in_=osb)
    return nc


@functools.lru_cache(maxsize=None)
def _build(G, finalize=True):
    nc = bacc.Bacc("TRN2", target_bir_lowering=False, debug=False,
                   num_swdge_queues=NQ)
    _emit(nc, G)
    if finalize:
        nc.finalize()
    return nc


def _make_idx(kept_local, G):
    """int16 gather-index tensor in the SWDGE layout: value for linear index
    i lives at [i % 16, i // 16], replicated to all 8 Q7 core groups."""
    n = kept_local.shape[0]
    a16 = kept_local.reshape(n // 16, 16).T.astype(np.int16)
    return np.tile(a16, (8, 1))


def _prep_core_inputs(s, t, kept, G):
    idx_local = (kept.reshape(G, NA) +
                 (np.arange(G, dtype=np.int64) * NO)[:, None]).reshape(-1)
    assert idx_local.max() < 32768
    B = G * NA // 128
    s_pm = np.ascontiguousarray(
        np.asarray(s, dtype=np.float32).reshape(B, 128, D).transpose(1, 0, 2))
    return {
        "s": s_pm,
        "t": np.ascontiguousarray(t, dtype=np.float32),
        "idx": _make_idx(idx_local.astype(np.int16), G),
        "ident": np.eye(128, dtype=ml_dtypes.bfloat16),
    }


def _ensure_profile_hook():
    """The trimmed container's antenv lacks axon_hooks; recreate it and
    register the ctypes NTFF hook so trace=True works (dev/test only)."""
    import sys
    import types
    try:
        from antenv.axon_hooks import get_axon_ntff_profile_hook  # noqa: F401
        return
    except ImportError:
        pass
    import antenv
    mod = types.ModuleType("antenv.axon_hooks")
    mod._hook = None

    def set_axon_ntff_profile_hook(h):
        mod._hook = h

    def get_axon_ntff_profile_hook():
        return mod._hook

    mod.set_axon_ntff_profile_hook = set_axon_ntff_profile_hook
    mod.get_axon_ntff_profile_hook = get_axon_ntff_profile_hook
    sys.modules["antenv.axon_hooks"] = mod
    antenv.axon_hooks = mod
    try:
        sys.path.insert(0, "/root/.axon_site")
        from trn_agent_boot.trn_boot import _ntff_profile_via_ctypes
        hook = _ntff_profile_via_ctypes("/opt/axon/libaxon_pjrt.so")
        if hook is not None:
            mod._hook = hook
    except Exception:
        pass


def run_device(inputs, trace=False, trace_cores=None, **kw):
    """Shard inputs, run on 8 cores, return (loss, BassKernelResults)."""
    s_all = np.asarray(inputs["student_nodes"], dtype=np.float32)
    t_all = np.asarray(inputs["teacher_nodes"], dtype=np.float32)
    kept = np.asarray(inputs["kept_node_indices"]).astype(np.int64)
    g_tot = int(np.asarray(inputs.get("num_graphs", 256)))
    na = int(np.asarray(inputs.get("n_aug", NA)))
    no = int(np.asarray(inputs.get("n_orig", NO)))
    assert (na, no, s_all.shape[-1]) == (NA, NO, D)
    assert g_tot % N_CORES == 0
    G = g_tot // N_CORES

    in_maps = []
    for k in range(N_CORES):
        in_maps.append(_prep_core_inputs(
            s_all[k * G * NA:(k + 1) * G * NA],
            t_all[k * G * NO:(k + 1) * G * NO],
            kept[k * G * NA:(k + 1) * G * NA],
            G,
        ))

    if trace:
        _ensure_profile_hook()
    nc = _build(G)
    res = run_bass_kernel_spmd(
        nc, in_maps, core_ids=list(range(N_CORES)),
        trace=trace, trace_cores=trace_cores, **kw,
    )
    total = sum(float(res.results[k]["out"][0, 0]) for k in range(N_CORES))
    loss = np.float32(total / (g_tot * na))
    return loss, res


def kernel(**inputs):
    loss, _ = run_device(inputs)
    return loss
